# revision 25
# baseline (speedup 1.0000x reference)
"""Causal multi-head attention (B=4, S=2048, D=1024, H=16) on 8 TRN2 NeuronCores.

Sharding: 4 batches x 2 head-groups (8 heads each) -> 8 cores.
All matmul inputs are bf16 (fp32 accumulation in PSUM).

Per core:
  - project tokens through the head-group's Wq/Wk/Wv columns into transposed
    [head_dim, token] layout (no on-device transposes needed),
  - causal attention (mask = tril(k=1): one future token allowed) for 8 heads
    as 4 head-pairs; QK^T runs both heads of a pair CONCURRENTLY in the two
    64-row halves of the PE array via tile_position row tiling; scoresT blocks
    [k, q] are exponentiated on the scalar engine; diagonal blocks are
    narrowed to the live query columns and masked with {0,1} bf16 multiplies
    on the vector engine (only the <=2 subblocks that need it),
  - softmax denominators come from a ones-column appended to V so the PV
    matmul accumulates both ctx^T and the exp-sums; per query-chunk all 8
    denominator rows are inverted with one reciprocal_approx_fast and
    partition-broadcast with a single DRAM round trip; the normalize
    multiplies run on the (otherwise idle) GpSimd engine so the vector/tensor
    queues never wait on the broadcast latency,
  - per query-chunk: output projection partial = ctx_part @ Wo_part + bo/2
    (emitted one chunk late so the tensor queue never stalls on normalize),
    then a chunked ReduceScatter(add) over the 2 cores of each batch overlaps
    the collective with the next chunk's attention compute,
  - emission is software-pipelined: x-chunks 0,1 project first, then chunk-0
    attention (scalar-engine exp work) overlaps the remaining projections.
"""

import numpy as np

B, S, D = 4, 2048, 1024
H = 16
HD = D // H  # 64
G = 2  # head groups (tensor-parallel degree per batch)
HPG = H // G  # 8 heads per core
DG = D // G  # 512 dims per group
P = 128
NKT = D // P  # 8 k-tiles over d_model
NQC = S // 512  # 4 query chunks of 512
NTT = S // P  # 16 token tiles of 128
NR = DG // P  # 4 head-pairs per group

_CACHE = {}


def _build_masks():
    """masks[s] is the [128, 512] multiplicative mask for a scoresT block
    [k_local, q_chunk_local] whose k-block index is kb = 4*qc + s.
    Allowed iff global k <= global q + 1."""
    masks = np.zeros((5, P, 512), dtype=np.float32)
    i = np.arange(P)[:, None]  # k local
    jj = np.arange(P)[None, :]  # q local within 128-subblock
    for s in range(5):
        for j in range(4):  # q subblock within the 512 chunk
            blk = masks[s][:, 128 * j : 128 * (j + 1)]
            if j > s:
                blk[:] = 1.0
            elif j == s:
                blk[:] = (i <= jj + 1).astype(np.float32)
            elif j == s - 1:
                blk[0, 127] = 1.0
    return masks


def _build_bass(collective=True):
    import concourse.bacc as bacc
    import concourse.mybir as mybir
    import concourse.tile as tile

    f32 = mybir.dt.float32
    bf16 = mybir.dt.bfloat16
    AF = mybir.ActivationFunctionType

    nc = bacc.Bacc("TRN2", target_bir_lowering=False, debug=False, num_devices=8)

    xT = nc.dram_tensor("xT", [D, S], bf16, kind="ExternalInput").ap()
    wq = nc.dram_tensor("wq", [D, DG], bf16, kind="ExternalInput").ap()
    wk = nc.dram_tensor("wk", [D, DG], bf16, kind="ExternalInput").ap()
    wv = nc.dram_tensor("wv", [D, DG], bf16, kind="ExternalInput").ap()
    wo = nc.dram_tensor("wo", [DG, D], bf16, kind="ExternalInput").ap()
    bo_b = nc.dram_tensor("bo_b", [P, D], f32, kind="ExternalInput").ap()
    masks = nc.dram_tensor("masks", [5, P, 512], bf16, kind="ExternalInput").ap()
    out_ext = nc.dram_tensor("out", [S // 2, D], bf16, kind="ExternalOutput").ap()

    with tile.TileContext(nc) as tc:
        with (
            tc.tile_pool(name="pqk", bufs=1) as pqk,
            tc.tile_pool(name="pv", bufs=1) as pv,
            tc.tile_pool(name="pc", bufs=1) as pc,
            tc.tile_pool(name="pmask", bufs=1) as pmask,
            tc.tile_pool(name="pw2", bufs=1) as pw2,
            tc.tile_pool(name="pw", bufs=3) as pw,
            tc.tile_pool(name="px", bufs=2) as px,
            tc.tile_pool(name="pe", bufs=4) as pe,
            tc.tile_pool(name="pn", bufs=2) as pn,
            tc.tile_pool(name="prcb", bufs=2) as prcb,
            tc.tile_pool(name="po_sb", bufs=2) as po_sb,
            tc.tile_pool(name="pdram", bufs=1, space="DRAM") as pdram,
            tc.tile_pool(name="pdram2", bufs=2, space="DRAM") as pdram2,
            # PSUM budget (8 banks): pp 2 (proj + out-proj accumulators),
            # psF 4 ([P,1024] score tiles, double-buffered), psC 2 (ctx0/ctx1).
            tc.tile_pool(name="pp", bufs=2, space="PSUM") as pp,
            tc.tile_pool(name="psF", bufs=2, space="PSUM") as psF,
            tc.tile_pool(name="psC", bufs=1, space="PSUM") as psC,
        ):
            # persistent SBUF tensors (all bf16)
            qT_sb = pqk.tile([P, NR, S], bf16)  # [dims of pair r | token]
            kT_sb = pqk.tile([P, NR, S], bf16)
            va_sb = pv.tile([P, NTT, HPG, HD + 1], bf16)  # v + ones col
            ctxT_sb = pc.tile([P, NR, S], bf16)
            masks_sb = pmask.tile([P, 5, 512], bf16)
            nc.sync.dma_start(masks_sb[:], masks.rearrange("s p q -> p s q"))
            # ones column of va: masks[s=0] block j=3 is all 1.0 (j > s);
            # memset can't encode a bf16 immediate, so copy ones from there.
            nc.vector.tensor_copy(
                va_sb[:, :, :, HD : HD + 1],
                masks_sb[:, 0, 384:512].rearrange("p (a b) -> p a b", b=HPG)[
                    :, :, :, None
                ],
            )

            wo_sb = pw2.tile([P, NR, D], bf16)
            nc.sync.dma_start(wo_sb[:], wo.rearrange("(ko p) f -> p ko f", p=P))
            bo_sb = pw2.tile([P, D], f32)
            nc.sync.dma_start(bo_sb[:], bo_b[:])

            partial = pdram.tile([S, D], bf16)
            rs_out = pdram.tile([S // 2, D], bf16)

            w_sbs = {}
            for name, w in (("wq", wq), ("wk", wk), ("wv", wv)):
                w_sb = pw.tile([P, NKT, DG], bf16, name=f"w_{name}", tag="w")
                nc.sync.dma_start(w_sb[:], w.rearrange("(ko p) f -> p ko f", p=P))
                w_sbs[name] = w_sb
            xT_r = xT.rearrange("(ko p) t -> p ko t", p=P)

            def proj_chunk(t):
                tok = slice(512 * t, 512 * (t + 1))
                xtile = px.tile([P, NKT, 512], bf16, name="xtile", tag="x")
                nc.sync.dma_start(xtile[:], xT_r[:, :, tok])
                # qT / kT: out [dims(pair r), 512 tokens]
                for name, dst in (("wq", qT_sb), ("wk", kT_sb)):
                    w_sb = w_sbs[name]
                    for rr in range(NR):
                        ps = pp.tile([P, 512], f32, name="ps_proj", tag="pp")
                        for kt in range(NKT):
                            nc.tensor.matmul(
                                ps[:],
                                w_sb[:, kt, P * rr : P * (rr + 1)],
                                xtile[:, kt, :],
                                start=(kt == 0),
                                stop=(kt == NKT - 1),
                            )
                        # scalar engine is mostly idle during projections
                        nc.scalar.copy(dst[:, rr, tok], ps[:])
                # v: out [128 tokens, 512 dims] per token tile
                w_sb = w_sbs["wv"]
                for st in range(4):
                    tt = 4 * t + st
                    ps = pp.tile([P, 512], f32, name="ps_v", tag="pp")
                    for kt in range(NKT):
                        nc.tensor.matmul(
                            ps[:],
                            xtile[:, kt, 128 * st : 128 * (st + 1)],
                            w_sb[:, kt, :],
                            start=(kt == 0),
                            stop=(kt == NKT - 1),
                        )
                    nc.vector.tensor_copy(
                        va_sb[:, tt, :, 0:HD],
                        ps[:].rearrange("p (h d) -> p h d", d=HD),
                    )

            def attn_chunk(qc, after_pr0=None):
                qs = slice(512 * qc, 512 * (qc + 1))
                nfull = 4 * qc
                smax = 4 if qc < 3 else 3
                last_kb = 4 * qc + smax
                # last chunk normalizes per head-pair to shorten the
                # end-of-kernel dependency chain
                per_pr_norm = qc == NQC - 1
                # engine writes must start at a 32-aligned partition, so the 8
                # denominator rows are collected along partition 0's free dim
                if not per_pr_norm:
                    den_sb = pn.tile([1, 2 * NR * 512], f32, name="den", tag="den")
                for pr in range(NR):
                    ctxs = [
                        psC.tile([HD + 1, 512], f32, name=f"ctx{hl}", tag=f"ctx{hl}")
                        for hl in range(2)
                    ]

                    def block(kb, qlo):
                        """One [128 k, 512-qlo q] scoresT block for both heads."""
                        ks = slice(128 * kb, 128 * (kb + 1))
                        W = 512 - qlo
                        sc = psF.tile([P, 1024], f32, name="sc", tag="sc")
                        et = pe.tile([P, 1024], bf16, name="et", tag="et")
                        for hl in range(2):
                            pp64 = slice(64 * hl, 64 * (hl + 1))
                            nc.tensor.matmul(
                                sc[:, 512 * hl : 512 * hl + W],
                                kT_sb[pp64, pr, ks],
                                qT_sb[pp64, pr, 512 * qc + qlo : 512 * (qc + 1)],
                                start=True,
                                stop=True,
                                tile_position=(64 * hl, 0),
                            )
                        if qlo == 0:
                            nc.scalar.activation(
                                et[:], sc[:], AF.Exp, scale=1.0 / 8.0
                            )
                        else:
                            sc_v = sc[:].rearrange("p (a b) -> p a b", b=512)
                            et_v = et[:].rearrange("p (a b) -> p a b", b=512)
                            nc.scalar.activation(
                                et_v[:, :, 0:W],
                                sc_v[:, :, 0:W],
                                AF.Exp,
                                scale=1.0 / 8.0,
                            )
                        return et

                    # PV runs one block late so the tensor queue never waits
                    # on the scalar engine's exp of the current block
                    pend = []

                    def flush_pv():
                        if pend:
                            kb, et, qlo = pend.pop()
                            for hl in range(2):
                                nc.tensor.matmul(
                                    ctxs[hl][:, qlo:512],
                                    va_sb[:, kb, 2 * pr + hl, :],
                                    et[:, 512 * hl : 512 * hl + (512 - qlo)],
                                    start=(kb == 0),
                                    stop=(kb == last_kb),
                                )

                    # full (unmasked) k-blocks
                    for kb in range(nfull):
                        et = block(kb, 0)
                        flush_pv()
                        pend.append((kb, et, 0))
                    # diagonal blocks, narrowed to live query columns
                    for s_ in range(smax + 1):
                        kb = 4 * qc + s_
                        qlo = 128 * (s_ - 1) if s_ >= 1 else 0
                        et = block(kb, qlo)
                        # mask subblocks j=s-1 (one live element) and j=s
                        # (triangular); j>s is all-ones -> no multiply
                        mhi = min(512, 128 * (s_ + 1))
                        for hl in range(2):
                            nc.vector.tensor_mul(
                                et[:, 512 * hl : 512 * hl + (mhi - qlo)],
                                et[:, 512 * hl : 512 * hl + (mhi - qlo)],
                                masks_sb[:, s_, qlo:mhi],
                            )
                        flush_pv()
                        pend.append((kb, et, qlo))
                    flush_pv()
                    # evacuate: unnormalized ctx^T (bf16) + denominator row
                    for hl in range(2):
                        nc.vector.tensor_copy(
                            ctxT_sb[64 * hl : 64 * (hl + 1), pr, qs],
                            ctxs[hl][0:HD, :],
                        )
                        if per_pr_norm:
                            continue
                        idx = 2 * pr + hl
                        nc.vector.tensor_copy(
                            den_sb[0:1, 512 * idx : 512 * (idx + 1)],
                            ctxs[hl][HD : HD + 1, :],
                        )
                    if per_pr_norm:
                        # per-pair normalize: collect, invert, broadcast, and
                        # multiply immediately (vector engine; it idles here)
                        den_p = pn.tile([1, 1024], f32, name="den3", tag="den3")
                        for hl in range(2):
                            nc.vector.tensor_copy(
                                den_p[0:1, 512 * hl : 512 * (hl + 1)],
                                ctxs[hl][HD : HD + 1, :],
                            )
                        rc_p = pn.tile([1, 1024], f32, name="rc3", tag="rc3")
                        nc.vector.reciprocal_approx_fast(rc_p[:], den_p[:])
                        rc16_p = pn.tile([1, 1024], bf16, name="rc163", tag="rc163")
                        nc.vector.tensor_copy(rc16_p[:], rc_p[:])
                        dd = pdram2.tile([1, 1024], bf16, name="dd3", tag="dd3")
                        nc.sync.dma_start(dd[:], rc16_p[:])
                        rcb_p = prcb.tile([P, 1024], bf16, name="rcb3", tag="rcb3")
                        nc.sync.dma_start(rcb_p[:], dd[:].to_broadcast((P, 1024)))
                        for hl in range(2):
                            sl = slice(64 * hl, 64 * (hl + 1))
                            nc.vector.tensor_mul(
                                ctxT_sb[sl, pr, qs],
                                ctxT_sb[sl, pr, qs],
                                rcb_p[sl, 512 * hl : 512 * (hl + 1)],
                            )
                    if pr == 0 and after_pr0 is not None:
                        after_pr0()

                if per_pr_norm:
                    return
                # ---- batched softmax normalization for this chunk ----
                # DRAM round trip 1: reshape the collected [1, 8*512] rows to
                # [8, 512] so the reciprocal runs on 8 lanes instead of 1
                den_d1 = pdram2.tile(
                    [1, 2 * NR * 512], f32, name="den_d1", tag="den_d1"
                )
                nc.sync.dma_start(den_d1[:], den_sb[:])
                den8 = pn.tile([2 * NR, 512], f32, name="den8", tag="den8")
                nc.sync.dma_start(
                    den8[:], den_d1[:].rearrange("a (b c) -> (a b) c", c=512)
                )
                rc = pn.tile([2 * NR, 512], f32, name="rc", tag="rc")
                nc.vector.reciprocal_approx_fast(rc[:], den8[:])
                rc16 = pn.tile([2 * NR, 512], bf16, name="rc16", tag="rc16")
                nc.vector.tensor_copy(rc16[:], rc[:])
                # DRAM round trip 2: partition-broadcast the reciprocals
                den_d = pdram2.tile([1, 2 * NR, 512], bf16, name="den_d", tag="den_d")
                nc.sync.dma_start(den_d[0], rc16[:])
                rcb = prcb.tile([P, 2 * NR, 512], bf16, name="rcb", tag="rcb", bufs=1)
                nc.sync.dma_start(rcb[:], den_d[:].to_broadcast((P, 2 * NR, 512)))
                # normalize on GpSimd: the idle engine waits on the broadcast
                # latency instead of Vector/Tensor
                for pr in range(NR):
                    for hl in range(2):
                        sl = slice(64 * hl, 64 * (hl + 1))
                        nc.gpsimd.tensor_mul(
                            ctxT_sb[sl, pr, qs],
                            ctxT_sb[sl, pr, qs],
                            rcb[sl, 2 * pr + hl, :],
                        )

            def rs_piece(pi, ri, rows):
                """ReduceScatter partial[pi:pi+rows] -> rs_out[ri:ri+rows//2]."""
                nc.gpsimd.collective_compute(
                    "ReduceScatter",
                    mybir.AluOpType.add,
                    replica_groups=[[0, 1], [2, 3], [4, 5], [6, 7]],
                    ins=[partial[pi : pi + rows, :].opt()],
                    outs=[rs_out[ri : ri + rows // 2, :].opt()],
                )
                nc.sync.dma_start(
                    out_ext[ri : ri + rows // 2, :], rs_out[ri : ri + rows // 2, :]
                )

            def outproj_rs(qc):
                # output projection for this chunk's 4 token tiles; the last
                # chunk ReduceScatters per token tile so only a 128-row
                # collective remains after the final matmul
                fine = qc == NQC - 1
                for st in range(4):
                    tt = 4 * qc + st
                    ts_ = slice(128 * tt, 128 * (tt + 1))
                    for nch in range(2):
                        ns = slice(512 * nch, 512 * (nch + 1))
                        ps = pp.tile([P, 512], f32, name="ps_o", tag="pp")
                        for rr in range(NR):
                            nc.tensor.matmul(
                                ps[:],
                                ctxT_sb[:, rr, ts_],
                                wo_sb[:, rr, ns],
                                start=(rr == 0),
                                stop=(rr == NR - 1),
                            )
                        ot = po_sb.tile([P, 512], bf16, name="ot", tag="ot")
                        nc.vector.tensor_add(ot[:], ps[:], bo_sb[:, ns])
                        nc.sync.dma_start(partial[ts_, ns], ot[:])
                    if collective and fine:
                        rs_piece(128 * tt, 256 * qc + 64 * st, 128)
                if collective and not fine:
                    for h in range(2):
                        rs_piece(512 * qc + 256 * h, 256 * qc + 128 * h, 256)

            # software-pipelined emission: attention chunks start as soon as
            # their K/V projections exist, filling the scalar engine during
            # the projection phase; out-proj of chunk qc is emitted after the
            # first head-pair of chunk qc+1 so its ReduceScatter overlaps
            # attention compute
            proj_chunk(0)
            proj_chunk(1)
            attn_chunk(0)
            proj_chunk(2)
            attn_chunk(1, after_pr0=lambda: outproj_rs(0))
            proj_chunk(3)
            attn_chunk(2, after_pr0=lambda: outproj_rs(1))
            attn_chunk(3, after_pr0=lambda: outproj_rs(2))
            outproj_rs(3)

            if not collective:
                nc.sync.dma_start(out_ext[:], partial[0 : S // 2, :])

    nc.compile()
    return nc


def _in_maps(x, Wq, Wk, Wv, Wo, bo):
    import ml_dtypes

    bf = ml_dtypes.bfloat16
    masks = _build_masks().astype(bf)
    maps = []
    for c in range(8):
        b, g = c // 2, c % 2
        cols = slice(DG * g, DG * (g + 1))
        maps.append(
            {
                "xT": np.ascontiguousarray(np.asarray(x)[b].T).astype(bf),
                "wq": np.ascontiguousarray(np.asarray(Wq)[:, cols]).astype(bf),
                "wk": np.ascontiguousarray(np.asarray(Wk)[:, cols]).astype(bf),
                "wv": np.ascontiguousarray(np.asarray(Wv)[:, cols]).astype(bf),
                "wo": np.ascontiguousarray(np.asarray(Wo)[cols, :]).astype(bf),
                "bo_b": np.broadcast_to(
                    np.asarray(bo, dtype=np.float32) / G, (P, D)
                ).copy(),
                "masks": masks,
            }
        )
    return maps


def _get_nc():
    if "nc" not in _CACHE:
        _CACHE["nc"] = _build_bass()
    return _CACHE["nc"]


def run(inputs, trace=False):
    from concourse.bass_utils import run_bass_kernel_spmd

    nc = _get_nc()
    maps = _in_maps(**inputs)
    res = run_bass_kernel_spmd(nc, maps, list(range(8)), trace=trace)
    out = np.empty((B, S, D), dtype=np.float32)
    for c in range(8):
        b, g = c // 2, c % 2
        r = res.results[c]["out"]  # [S//2, D] bf16, RS piece layout (see build)
        for qc in range(NQC - 1):
            for h in range(2):
                dst = 512 * qc + 256 * h + 128 * g
                src = 256 * qc + 128 * h
                out[b, dst : dst + 128, :] = r[src : src + 128, :].astype(np.float32)
        for st in range(4):  # last chunk: per-token-tile pieces
            dst = 1536 + 128 * st + 64 * g
            src = 768 + 64 * st
            out[b, dst : dst + 64, :] = r[src : src + 64, :].astype(np.float32)
    return out, res


def kernel(x, Wq, Wk, Wv, Wo, bo):
    out, _ = run(dict(x=x, Wq=Wq, Wk=Wk, Wv=Wv, Wo=Wo, bo=bo))
    return out


# revision 31
# speedup vs baseline: 1.0514x; 1.0514x over previous
"""Causal multi-head attention (B=4, S=2048, D=1024, H=16) on 8 TRN2 NeuronCores.

Sharding: 4 batches x 2 head-groups (8 heads each) -> 8 cores.
All matmul inputs are bf16 (fp32 accumulation in PSUM).

Per core:
  - project tokens through the head-group's Wq/Wk/Wv columns into transposed
    [head_dim, token] layout (no on-device transposes needed),
  - causal attention (mask = tril(k=1): one future token allowed) for 8 heads
    as 4 head-pairs; QK^T runs both heads of a pair CONCURRENTLY in the two
    64-row halves of the PE array via tile_position row tiling; scoresT blocks
    [k, q] are exponentiated on the scalar engine; diagonal blocks are
    narrowed to the live query columns and masked with {0,1} bf16 multiplies
    on the vector engine (only the <=2 subblocks that need it),
  - softmax denominators come from a ones-column appended to V so the PV
    matmul accumulates both ctx^T and the exp-sums; per query-chunk all 8
    denominator rows are inverted with one reciprocal_approx_fast and
    partition-broadcast with a single DRAM round trip; the normalize
    multiplies run on the (otherwise idle) GpSimd engine so the vector/tensor
    queues never wait on the broadcast latency,
  - per query-chunk: output projection partial = ctx_part @ Wo_part + bo/2
    (emitted one chunk late so the tensor queue never stalls on normalize),
    then a chunked ReduceScatter(add) over the 2 cores of each batch overlaps
    the collective with the next chunk's attention compute,
  - emission is software-pipelined: x-chunks 0,1 project first, then chunk-0
    attention (scalar-engine exp work) overlaps the remaining projections.
"""

import numpy as np

B, S, D = 4, 2048, 1024
H = 16
HD = D // H  # 64
G = 2  # head groups (tensor-parallel degree per batch)
HPG = H // G  # 8 heads per core
DG = D // G  # 512 dims per group
P = 128
NKT = D // P  # 8 k-tiles over d_model
NQC = S // 512  # 4 query chunks of 512
NTT = S // P  # 16 token tiles of 128
NR = DG // P  # 4 head-pairs per group

_CACHE = {}


def _build_masks():
    """masks[s] is the [128, 512] multiplicative mask for a scoresT block
    [k_local, q_chunk_local] whose k-block index is kb = 4*qc + s.
    Allowed iff global k <= global q + 1."""
    masks = np.zeros((5, P, 512), dtype=np.float32)
    i = np.arange(P)[:, None]  # k local
    jj = np.arange(P)[None, :]  # q local within 128-subblock
    for s in range(5):
        for j in range(4):  # q subblock within the 512 chunk
            blk = masks[s][:, 128 * j : 128 * (j + 1)]
            if j > s:
                blk[:] = 1.0
            elif j == s:
                blk[:] = (i <= jj + 1).astype(np.float32)
            elif j == s - 1:
                blk[0, 127] = 1.0
    return masks


def _build_bass(collective=True):
    import concourse.bacc as bacc
    import concourse.mybir as mybir
    import concourse.tile as tile

    f32 = mybir.dt.float32
    bf16 = mybir.dt.bfloat16
    AF = mybir.ActivationFunctionType

    nc = bacc.Bacc("TRN2", target_bir_lowering=False, debug=False, num_devices=8)

    xT = nc.dram_tensor("xT", [D, S], bf16, kind="ExternalInput").ap()
    wq = nc.dram_tensor("wq", [D, DG], bf16, kind="ExternalInput").ap()
    wk = nc.dram_tensor("wk", [D, DG], bf16, kind="ExternalInput").ap()
    wv = nc.dram_tensor("wv", [D, DG], bf16, kind="ExternalInput").ap()
    wo = nc.dram_tensor("wo", [DG, D], bf16, kind="ExternalInput").ap()
    bo_b = nc.dram_tensor("bo_b", [P, D], f32, kind="ExternalInput").ap()
    masks = nc.dram_tensor("masks", [5, P, 512], bf16, kind="ExternalInput").ap()
    out_ext = nc.dram_tensor("out", [S // 2, D], bf16, kind="ExternalOutput").ap()

    with tile.TileContext(nc) as tc:
        with (
            tc.tile_pool(name="pqk", bufs=1) as pqk,
            tc.tile_pool(name="pv", bufs=1) as pv,
            tc.tile_pool(name="pc", bufs=1) as pc,
            tc.tile_pool(name="pmask", bufs=1) as pmask,
            tc.tile_pool(name="pw2", bufs=1) as pw2,
            tc.tile_pool(name="pw", bufs=3) as pw,
            tc.tile_pool(name="px", bufs=2) as px,
            tc.tile_pool(name="pe", bufs=4) as pe,
            tc.tile_pool(name="pn", bufs=2) as pn,
            tc.tile_pool(name="prcb", bufs=2) as prcb,
            tc.tile_pool(name="po_sb", bufs=2) as po_sb,
            tc.tile_pool(name="pdram", bufs=1, space="DRAM") as pdram,
            tc.tile_pool(name="pdram2", bufs=2, space="DRAM") as pdram2,
            # PSUM budget (8 banks): pp 2 (proj + out-proj accumulators),
            # psF 4 ([P,1024] score tiles, double-buffered), psC 2 (ctx0/ctx1).
            tc.tile_pool(name="pp", bufs=2, space="PSUM") as pp,
            tc.tile_pool(name="psF", bufs=2, space="PSUM") as psF,
            tc.tile_pool(name="psC", bufs=1, space="PSUM") as psC,
        ):
            # persistent SBUF tensors (all bf16)
            qT_sb = pqk.tile([P, NR, S], bf16)  # [dims of pair r | token]
            kT_sb = pqk.tile([P, NR, S], bf16)
            va_sb = pv.tile([P, NTT, HPG, HD + 1], bf16)  # v + ones col
            ctxT_sb = pc.tile([P, NR, S], bf16)
            masks_sb = pmask.tile([P, 5, 512], bf16)
            nc.sync.dma_start(masks_sb[:], masks.rearrange("s p q -> p s q"))
            # ones column of va: masks[s=0] block j=3 is all 1.0 (j > s);
            # memset can't encode a bf16 immediate, so copy ones from there.
            nc.vector.tensor_copy(
                va_sb[:, :, :, HD : HD + 1],
                masks_sb[:, 0, 384:512].rearrange("p (a b) -> p a b", b=HPG)[
                    :, :, :, None
                ],
            )

            wo_sb = pw2.tile([P, NR, D], bf16)
            nc.sync.dma_start(wo_sb[:], wo.rearrange("(ko p) f -> p ko f", p=P))
            bo_sb = pw2.tile([P, D], f32)
            nc.sync.dma_start(bo_sb[:], bo_b[:])

            partial = pdram.tile([S, D], bf16)
            rs_out = pdram.tile([S // 2, D], bf16)

            w_sbs = {}
            for name, w in (("wq", wq), ("wk", wk), ("wv", wv)):
                w_sb = pw.tile([P, NKT, DG], bf16, name=f"w_{name}", tag="w")
                nc.sync.dma_start(w_sb[:], w.rearrange("(ko p) f -> p ko f", p=P))
                w_sbs[name] = w_sb
            xT_r = xT.rearrange("(ko p) t -> p ko t", p=P)

            def proj_chunk(t):
                tok = slice(512 * t, 512 * (t + 1))
                xtile = px.tile([P, NKT, 512], bf16, name="xtile", tag="x")
                nc.sync.dma_start(xtile[:], xT_r[:, :, tok])
                # qT / kT: out [dims(pair r), 512 tokens]
                for name, dst in (("wq", qT_sb), ("wk", kT_sb)):
                    w_sb = w_sbs[name]
                    for rr in range(NR):
                        ps = pp.tile([P, 512], f32, name="ps_proj", tag="pp")
                        for kt in range(NKT):
                            nc.tensor.matmul(
                                ps[:],
                                w_sb[:, kt, P * rr : P * (rr + 1)],
                                xtile[:, kt, :],
                                start=(kt == 0),
                                stop=(kt == NKT - 1),
                            )
                        # scalar engine is mostly idle during projections
                        nc.scalar.copy(dst[:, rr, tok], ps[:])
                # v: out [128 tokens, 512 dims] per token tile
                w_sb = w_sbs["wv"]
                for st in range(4):
                    tt = 4 * t + st
                    ps = pp.tile([P, 512], f32, name="ps_v", tag="pp")
                    for kt in range(NKT):
                        nc.tensor.matmul(
                            ps[:],
                            xtile[:, kt, 128 * st : 128 * (st + 1)],
                            w_sb[:, kt, :],
                            start=(kt == 0),
                            stop=(kt == NKT - 1),
                        )
                    nc.vector.tensor_copy(
                        va_sb[:, tt, :, 0:HD],
                        ps[:].rearrange("p (h d) -> p h d", d=HD),
                    )

            def attn_chunk(qc, after_pr0=None):
                qs = slice(512 * qc, 512 * (qc + 1))
                nfull = 4 * qc
                smax = 4 if qc < 3 else 3
                last_kb = 4 * qc + smax
                # last chunk normalizes per head-pair to shorten the
                # end-of-kernel dependency chain
                per_pr_norm = qc == NQC - 1
                # engine writes must start at a 32-aligned partition, so the 8
                # denominator rows are collected along partition 0's free dim
                if not per_pr_norm:
                    den_sb = pn.tile([1, 2 * NR * 512], f32, name="den", tag="den")
                for pr in range(NR):
                    ctxs = [
                        psC.tile([HD + 1, 512], f32, name=f"ctx{hl}", tag=f"ctx{hl}")
                        for hl in range(2)
                    ]

                    def block(kb, qlo):
                        """One [128 k, 512-qlo q] scoresT block for both heads."""
                        ks = slice(128 * kb, 128 * (kb + 1))
                        W = 512 - qlo
                        sc = psF.tile([P, 1024], f32, name="sc", tag="sc")
                        et = pe.tile([P, 1024], bf16, name="et", tag="et")
                        for hl in range(2):
                            pp64 = slice(64 * hl, 64 * (hl + 1))
                            nc.tensor.matmul(
                                sc[:, 512 * hl : 512 * hl + W],
                                kT_sb[pp64, pr, ks],
                                qT_sb[pp64, pr, 512 * qc + qlo : 512 * (qc + 1)],
                                start=True,
                                stop=True,
                                tile_position=(64 * hl, 0),
                            )
                        if qlo == 0:
                            nc.scalar.activation(
                                et[:], sc[:], AF.Exp, scale=1.0 / 8.0
                            )
                        else:
                            sc_v = sc[:].rearrange("p (a b) -> p a b", b=512)
                            et_v = et[:].rearrange("p (a b) -> p a b", b=512)
                            nc.scalar.activation(
                                et_v[:, :, 0:W],
                                sc_v[:, :, 0:W],
                                AF.Exp,
                                scale=1.0 / 8.0,
                            )
                        return et

                    # PV runs one block late so the tensor queue never waits
                    # on the scalar engine's exp of the current block
                    pend = []

                    def flush_pv():
                        if pend:
                            kb, et, qlo = pend.pop()
                            for hl in range(2):
                                nc.tensor.matmul(
                                    ctxs[hl][:, qlo:512],
                                    va_sb[:, kb, 2 * pr + hl, :],
                                    et[:, 512 * hl : 512 * hl + (512 - qlo)],
                                    start=(kb == 0),
                                    stop=(kb == last_kb),
                                )

                    # full (unmasked) k-blocks
                    for kb in range(nfull):
                        et = block(kb, 0)
                        flush_pv()
                        pend.append((kb, et, 0))
                    # diagonal blocks, narrowed to live query columns
                    for s_ in range(smax + 1):
                        kb = 4 * qc + s_
                        qlo = 128 * (s_ - 1) if s_ >= 1 else 0
                        et = block(kb, qlo)
                        # mask subblocks j=s-1 (one live element) and j=s
                        # (triangular); j>s is all-ones -> no multiply
                        mhi = min(512, 128 * (s_ + 1))
                        for hl in range(2):
                            nc.vector.tensor_mul(
                                et[:, 512 * hl : 512 * hl + (mhi - qlo)],
                                et[:, 512 * hl : 512 * hl + (mhi - qlo)],
                                masks_sb[:, s_, qlo:mhi],
                            )
                        flush_pv()
                        pend.append((kb, et, qlo))
                    flush_pv()
                    # evacuate: unnormalized ctx^T (bf16) + denominator row
                    for hl in range(2):
                        nc.vector.tensor_copy(
                            ctxT_sb[64 * hl : 64 * (hl + 1), pr, qs],
                            ctxs[hl][0:HD, :],
                        )
                        if per_pr_norm:
                            continue
                        idx = 2 * pr + hl
                        nc.vector.tensor_copy(
                            den_sb[0:1, 512 * idx : 512 * (idx + 1)],
                            ctxs[hl][HD : HD + 1, :],
                        )
                    if per_pr_norm:
                        # per-pair normalize: collect, invert, broadcast, and
                        # multiply immediately; den collection on the (idle)
                        # scalar engine overlaps the vector-engine evacuation
                        den_p = pn.tile([1, 1024], f32, name="den3", tag="den3")
                        for hl in range(2):
                            nc.scalar.copy(
                                den_p[0:1, 512 * hl : 512 * (hl + 1)],
                                ctxs[hl][HD : HD + 1, :],
                            )
                        rc_p = pn.tile([1, 1024], f32, name="rc3", tag="rc3")
                        nc.vector.reciprocal_approx_fast(rc_p[:], den_p[:])
                        rc16_p = pn.tile([1, 1024], bf16, name="rc163", tag="rc163")
                        nc.vector.tensor_copy(rc16_p[:], rc_p[:])
                        dd = pdram2.tile([1, 1024], bf16, name="dd3", tag="dd3")
                        nc.sync.dma_start(dd[:], rc16_p[:])
                        rcb_p = prcb.tile([P, 1024], bf16, name="rcb3", tag="rcb3")
                        nc.sync.dma_start(rcb_p[:], dd[:].to_broadcast((P, 1024)))
                        for hl in range(2):
                            sl = slice(64 * hl, 64 * (hl + 1))
                            nc.vector.tensor_mul(
                                ctxT_sb[sl, pr, qs],
                                ctxT_sb[sl, pr, qs],
                                rcb_p[sl, 512 * hl : 512 * (hl + 1)],
                            )
                    if pr == 0 and after_pr0 is not None:
                        after_pr0()

                if per_pr_norm:
                    return
                # ---- batched softmax normalization for this chunk ----
                # DRAM round trip 1: reshape the collected [1, 8*512] rows to
                # [8, 512] so the reciprocal runs on 8 lanes instead of 1
                den_d1 = pdram2.tile(
                    [1, 2 * NR * 512], f32, name="den_d1", tag="den_d1"
                )
                nc.sync.dma_start(den_d1[:], den_sb[:])
                den8 = pn.tile([2 * NR, 512], f32, name="den8", tag="den8")
                nc.sync.dma_start(
                    den8[:], den_d1[:].rearrange("a (b c) -> (a b) c", c=512)
                )
                rc = pn.tile([2 * NR, 512], f32, name="rc", tag="rc")
                nc.vector.reciprocal_approx_fast(rc[:], den8[:])
                rc16 = pn.tile([2 * NR, 512], bf16, name="rc16", tag="rc16")
                nc.vector.tensor_copy(rc16[:], rc[:])
                # DRAM round trip 2: partition-broadcast the reciprocals
                den_d = pdram2.tile([1, 2 * NR, 512], bf16, name="den_d", tag="den_d")
                nc.sync.dma_start(den_d[0], rc16[:])
                rcb = prcb.tile([P, 2 * NR, 512], bf16, name="rcb", tag="rcb", bufs=1)
                nc.sync.dma_start(rcb[:], den_d[:].to_broadcast((P, 2 * NR, 512)))
                # normalize on GpSimd: the idle engine waits on the broadcast
                # latency instead of Vector/Tensor
                for pr in range(NR):
                    for hl in range(2):
                        sl = slice(64 * hl, 64 * (hl + 1))
                        nc.gpsimd.tensor_mul(
                            ctxT_sb[sl, pr, qs],
                            ctxT_sb[sl, pr, qs],
                            rcb[sl, 2 * pr + hl, :],
                        )

            def rs_piece(pi, ri, rows):
                """ReduceScatter partial[pi:pi+rows] -> rs_out[ri:ri+rows//2]."""
                nc.gpsimd.collective_compute(
                    "ReduceScatter",
                    mybir.AluOpType.add,
                    replica_groups=[[0, 1], [2, 3], [4, 5], [6, 7]],
                    ins=[partial[pi : pi + rows, :].opt()],
                    outs=[rs_out[ri : ri + rows // 2, :].opt()],
                )
                nc.sync.dma_start(
                    out_ext[ri : ri + rows // 2, :], rs_out[ri : ri + rows // 2, :]
                )

            def outproj_rs(qc):
                # output projection for this chunk's 4 token tiles
                for st in range(4):
                    tt = 4 * qc + st
                    ts_ = slice(128 * tt, 128 * (tt + 1))
                    for nch in range(2):
                        ns = slice(512 * nch, 512 * (nch + 1))
                        ps = pp.tile([P, 512], f32, name="ps_o", tag="pp")
                        for rr in range(NR):
                            nc.tensor.matmul(
                                ps[:],
                                ctxT_sb[:, rr, ts_],
                                wo_sb[:, rr, ns],
                                start=(rr == 0),
                                stop=(rr == NR - 1),
                            )
                        ot = po_sb.tile([P, 512], bf16, name="ot", tag="ot")
                        nc.vector.tensor_add(ot[:], ps[:], bo_sb[:, ns])
                        nc.sync.dma_start(partial[ts_, ns], ot[:])
                # chunked ReduceScatter (two halves), overlapping compute
                if collective:
                    for h in range(2):
                        rs_piece(512 * qc + 256 * h, 256 * qc + 128 * h, 256)

            # software-pipelined emission: chunk-0 attention (scalar-engine
            # work) overlaps the tail projections; out-proj runs a chunk late
            proj_chunk(0)
            proj_chunk(1)
            attn_chunk(0)
            proj_chunk(2)
            proj_chunk(3)
            attn_chunk(1)
            outproj_rs(0)
            attn_chunk(2)
            outproj_rs(1)
            attn_chunk(3)
            outproj_rs(2)
            outproj_rs(3)

            if not collective:
                nc.sync.dma_start(out_ext[:], partial[0 : S // 2, :])

    nc.compile()
    return nc


def _in_maps(x, Wq, Wk, Wv, Wo, bo):
    import ml_dtypes

    bf = ml_dtypes.bfloat16
    masks = _build_masks().astype(bf)
    maps = []
    for c in range(8):
        b, g = c // 2, c % 2
        cols = slice(DG * g, DG * (g + 1))
        maps.append(
            {
                "xT": np.ascontiguousarray(np.asarray(x)[b].T).astype(bf),
                "wq": np.ascontiguousarray(np.asarray(Wq)[:, cols]).astype(bf),
                "wk": np.ascontiguousarray(np.asarray(Wk)[:, cols]).astype(bf),
                "wv": np.ascontiguousarray(np.asarray(Wv)[:, cols]).astype(bf),
                "wo": np.ascontiguousarray(np.asarray(Wo)[cols, :]).astype(bf),
                "bo_b": np.broadcast_to(
                    np.asarray(bo, dtype=np.float32) / G, (P, D)
                ).copy(),
                "masks": masks,
            }
        )
    return maps


def _get_nc():
    if "nc" not in _CACHE:
        _CACHE["nc"] = _build_bass()
    return _CACHE["nc"]


def run(inputs, trace=False):
    from concourse.bass_utils import run_bass_kernel_spmd

    nc = _get_nc()
    maps = _in_maps(**inputs)
    res = run_bass_kernel_spmd(nc, maps, list(range(8)), trace=trace)
    out = np.empty((B, S, D), dtype=np.float32)
    for c in range(8):
        b, g = c // 2, c % 2
        r = res.results[c]["out"]  # [S//2, D] bf16: 8 half-chunks of 128 rows
        for qc in range(NQC):
            for h in range(2):
                dst = 512 * qc + 256 * h + 128 * g
                src = 256 * qc + 128 * h
                out[b, dst : dst + 128, :] = r[src : src + 128, :].astype(np.float32)
    return out, res


def kernel(x, Wq, Wk, Wv, Wo, bo):
    out, _ = run(dict(x=x, Wq=Wq, Wk=Wk, Wv=Wv, Wo=Wo, bo=bo))
    return out
